# revision 33
# baseline (speedup 1.0000x reference)
"""Trainium2 Bass kernel for multi-head attention (nn_Attention_39573828665669).

Reference computation (per batch b=1, SQ=SKV=2048, DM=2048, H=32, DH=64):
    q = hidden_q @ wq.T + bq ; k = hidden_kv @ wk.T + bk ; v = hidden_kv @ wv.T + bv
    score = q @ k.T / sqrt(DH) + position_bias ; masked softmax ; out = probs @ v
    return out @ wo.T + bo

Sharding: tensor-parallel over heads. Each of the 8 cores handles 4 heads
(256 of the 2048 head-dims), computes its partial output projection
out_c = attn_c @ wo[:, cols_c].T, and the host sums the 8 partials.

Device-side formulation (per core):
  - All matmuls run in fp16 (1 cycle/row on the PE; fp32r is ~24x slower on
    real HW). PSUM accumulation is fp32.
  - The 1/sqrt(DH) scale is folded into wq on the host.
  - mask and position_bias are folded on the host into
    E = exp(pb) * mask / 256 (fp16, transposed per head to [tk, tq]).
    Device computes P' = exp(S.T) * E with masked entries exactly 0.
  - Softmax runs unnormalized: V is stored with 64 extra all-ones columns
    per head, so the AV matmul's output rows 64:128 are the softmax
    denominator replicated across 64 partitions -- the partition broadcast
    costs nothing (it's the stationary-operand width). Normalization uses
    DVE only (ACT is the stage-2 bottleneck with 16.8M exps per core):
    one eager full-tile copy releases the AV psum banks for the next head,
    then denominator-bounce + reciprocal_approx_fast + multiply run
    deferred, interleaved into the next head's loop where DVE has slack.
    (Pitfalls: recip_approx silently corrupts on base-partition-64 inputs,
    and SB+SB tensor_tensor requires equal input base partitions -- hence
    the base-0 bounce copies.)
  - bv is folded on the host as out += bv @ wo.T; bo is added on the host.

Schedule (all under one TileContext; Tile overlaps stages via tile deps):
  - hidden_kv is loaded to SBUF once (8 x ~1MB DMAs) and serves both the
    K and V projections -- no HBM re-read.
  - Stage order V, Q, K: after each rep's loop barrier V runs on
    SBUF-resident hkv (no DMA) while Q's hq stream prefetches.
  - E and hq stream as 1MB double-chunk DMAs (halves the per-DMA fixed
    cost); the E pool lets the DMA run ~6 tk-chunks ahead of consumption.
  - Stage 2 per (head, tkc): 4 score matmuls (K=64), 2 exps of [128,1024]
    (ACT, PSUM-src), 2 fp16 multiplies by E (DVE 2x mode), AV matmuls
    issued one tkc behind so the in-order PE never waits on pp.
  - Stage 4 keeps wo stationary (output transposed to [DM, T]; the host
    transposes back after summing the 8 partials); PSUM evacs alternate
    ACT/DVE everywhere so no single engine serializes a stage boundary.

All HBM-touching DMAs move >= 4 KB per partition line; weights arrive
host-prepacked in their SBUF layout (one big DMA each).
"""

import os
from contextlib import ExitStack

import numpy as np

import concourse.bass as bass
import concourse.tile as tile
from concourse import bacc, mybir
from concourse.bass_utils import run_bass_kernel_spmd

F32 = mybir.dt.float32
FP16 = mybir.dt.float16

ts = bass.ts

N_CORES = 8
H = 32
DH = 64
HPC = H // N_CORES          # heads per core = 4
M = HPC * DH                # per-core head dims = 256
E_SCALE = 1.0 / 256.0       # keeps P' = exp(s + pb)/256 < ~2000 (fp16 max 65504)


def build_attention_nc(
    T: int, DM: int, reps: int = 1, with_bias: bool = False, parts: str = "all",
    av_pipe: bool = True, dve_norm: bool = True,
):
    """Build the per-core Bass program. SPMD: all cores run this NEFF."""
    assert HPC == 4 and DH == 64
    NJ = M // 128            # m chunks (2)
    NDC = DM // 128          # contraction chunks for projections
    NTK = T // 128           # key-position chunks
    NTQ = T // 512           # 512-wide query chunks
    TQB = 1024               # stage-2 query chunk width
    NTQB = T // TQB
    NKV = 8                  # hkv SBUF tiles (DMA granularity ~1MB)

    nc = bacc.Bacc("TRN2", target_bir_lowering=False, debug=False)

    d_hqT = nc.declare_dram_parameter("hqT", [DM, T], FP16, isOutput=False)
    # hkv prepacked on host into SBUF chunk layout [128, NDC, T]
    d_hkvP = nc.declare_dram_parameter("hkvP", [128, NDC * T], FP16, isOutput=False)
    # weights prepacked on host into SBUF layout (one big DMA each)
    d_wq = nc.declare_dram_parameter("wqT", [128, NDC * M], FP16, isOutput=False)
    d_wk = nc.declare_dram_parameter("wkT", [128, NDC * M], FP16, isOutput=False)
    d_wv = nc.declare_dram_parameter("wvT", [128, NDC * M], FP16, isOutput=False)
    d_wo = nc.declare_dram_parameter("woT", [128, NJ * DM], FP16, isOutput=False)
    d_bq = nc.declare_dram_parameter("bq", [M], F32, isOutput=False)
    d_bk = nc.declare_dram_parameter("bk", [M], F32, isOutput=False)
    d_E = nc.declare_dram_parameter("E", [HPC, T, T], FP16, isOutput=False)
    # output stored transposed [DM, T]: lets stage 4 keep wo stationary on
    # the PE (one LDWEIGHTS per (dm-chunk, j) streams all T columns)
    d_out = nc.declare_dram_parameter("out", [DM, T], FP16, isOutput=True)

    Exp = mybir.ActivationFunctionType.Exp
    Identity = mybir.ActivationFunctionType.Identity
    Ln = mybir.ActivationFunctionType.Ln

    with tile.TileContext(nc) as tc, ExitStack() as ctx:
        wpool = ctx.enter_context(tc.tile_pool(name="weights", bufs=1))
        spool = ctx.enter_context(tc.tile_pool(name="state", bufs=1))

        wv_sb = wpool.tile([128, NDC, M], FP16, tag="wv")
        wk_sb = wpool.tile([128, NDC, M], FP16, tag="wk")
        wq_sb = wpool.tile([128, NDC, M], FP16, tag="wq")
        wo_sb = wpool.tile([128, NJ, DM], FP16, tag="wo")
        bq_sb = wpool.tile([128, NJ], F32, tag="bq")
        bk_sb = wpool.tile([128, NJ], F32, tag="bk")
        # hkv resident in SBUF: serves K-proj (rhs rows) and V-proj (lhsT)
        hkv_sb = [
            wpool.tile([128, NDC // NKV, T], FP16, tag=f"hkv{i}", name=f"hkv{i}")
            for i in range(NKV)
        ]
        nc.sync.dma_start(wv_sb[:].rearrange("p a b -> p (a b)"), d_wv.ap())
        for i in range(NKV):
            nc.sync.dma_start(
                hkv_sb[i][:].rearrange("p a b -> p (a b)"),
                d_hkvP.ap()[:, i * (NDC // NKV) * T : (i + 1) * (NDC // NKV) * T],
            )
        nc.sync.dma_start(wq_sb[:].rearrange("p a b -> p (a b)"), d_wq.ap())
        nc.sync.dma_start(wk_sb[:].rearrange("p a b -> p (a b)"), d_wk.ap())
        nc.sync.dma_start(wo_sb[:].rearrange("p a b -> p (a b)"), d_wo.ap())
        nc.sync.dma_start(bq_sb[:], d_bq.ap().rearrange("(j p) -> p j", p=128))
        nc.sync.dma_start(bk_sb[:], d_bk.ap().rearrange("(j p) -> p j", p=128))

        def hkv(dc):
            return hkv_sb[dc // (NDC // NKV)][:, dc % (NDC // NKV), :]

        qt_sb = spool.tile([128, NJ, T], FP16, tag="qt")     # Q.T / sqrt(DH)
        kt_sb = spool.tile([128, NJ, T], FP16, tag="kt")     # K.T
        # V per tk-chunk: per head 64 V columns then 64 ones columns.
        # AV with lhsT = [V_h | 1s] gives PSUM rows 0:64 = O, 64:128 = denom
        # broadcast across 64 partitions for free.
        vo_sb = spool.tile([128, NTK, HPC * 128], FP16, tag="vo")
        ot_sb = spool.tile([128, NJ, T], FP16, tag="ot")     # normalized O.T

        ones_view = vo_sb[:].rearrange("p n (h x) -> p n h x", x=128)[
            :, :, :, DH : 2 * DH
        ]
        nc.vector.memset(ones_view, 1.0)

        def _evac(dst, src, b_sb, j, idx=0):
            # alternate ACT/DVE so the post-projection evac tail is short
            # (the next stage's PSUM pool waits on all releases)
            if with_bias:
                nc.scalar.activation(dst, src, Identity, bias=b_sb[:, j : j + 1])
            elif idx % 2 == 0:
                nc.scalar.activation(dst, src, mybir.ActivationFunctionType.Copy)
            else:
                nc.vector.tensor_copy(dst, src)

        def _proj_v():
            # V in [t, m] layout over SBUF-resident hkv. Two half-T passes:
            # each [128, M] psum tile pads to a full bank, and a bank cannot
            # host two accumulation groups (start=True clears has_written
            # for the whole bank), so 16 chunks don't fit in 8 banks at once.
            NH = NTK // 2
            for half in range(2):
                with tc.tile_pool(name="pv", bufs=8, space="PSUM") as pv:
                    vp = [
                        pv.tile([128, M], F32, tag="vp", name=f"vp{half}_{i}")
                        for i in range(NH)
                    ]
                    for dc in range(NDC):
                        hrow = hkv(dc)
                        for tsub in range(NH):
                            nc.tensor.matmul(
                                vp[tsub][:],
                                lhsT=hrow[:, ts(half * NH + tsub, 128)],
                                rhs=wv_sb[:, dc, :],
                                start=(dc == 0),
                                stop=(dc == NDC - 1),
                            )
                    for tsub in range(NH):
                        src = vp[tsub][:].rearrange("p (h d) -> p h d", d=DH)
                        dst = vo_sb[:, half * NH + tsub, :].rearrange(
                            "p (h x) -> p h x", x=128
                        )[:, :, 0:DH]
                        if tsub % 2 == 0:
                            nc.vector.tensor_copy(dst, src)
                        else:
                            nc.scalar.activation(
                                dst, src, mybir.ActivationFunctionType.Copy
                            )

        def _proj_k():
            # K from SBUF-resident hkv: full-T psum (8 banks).
            with tc.tile_pool(name="pk", bufs=8, space="PSUM") as pk:
                pp = [
                    pk.tile([128, 512], F32, tag="pp", name=f"ppk{i}")
                    for i in range(NJ * NTQ)
                ]
                for dc in range(NDC):
                    hrow = hkv(dc)
                    for j in range(NJ):
                        for q in range(NTQ):
                            nc.tensor.matmul(
                                pp[j * NTQ + q][:],
                                lhsT=wk_sb[:, dc, ts(j, 128)],
                                rhs=hrow[:, ts(q, 512)],
                                start=(dc == 0),
                                stop=(dc == NDC - 1),
                            )
                for j in range(NJ):
                    for q in range(NTQ):
                        _evac(
                            kt_sb[:, j, ts(q, 512)],
                            pp[j * NTQ + q][:],
                            bk_sb,
                            j,
                            j * NTQ + q,
                        )

        def _proj_q():
            # Q streams hidden_q from HBM (one [128, T] row-chunk per dc).
            with (
                tc.tile_pool(name="pq", bufs=8, space="PSUM") as pq,
                tc.tile_pool(name="hin", bufs=2) as hin,
            ):
                pp = [
                    pq.tile([128, 512], F32, tag="pp", name=f"ppq{i}")
                    for i in range(NJ * NTQ)
                ]
                hview = d_hqT.ap().rearrange("(a p) t -> p a t", p=128)
                ht = None
                for dc in range(NDC):
                    if dc % 2 == 0:
                        ht = hin.tile([128, 2, T], FP16, tag="h")
                        nc.sync.dma_start(ht[:], hview[:, dc : dc + 2, :])
                    for j in range(NJ):
                        for q in range(NTQ):
                            nc.tensor.matmul(
                                pp[j * NTQ + q][:],
                                lhsT=wq_sb[:, dc, ts(j, 128)],
                                rhs=ht[:, dc % 2, ts(q, 512)],
                                start=(dc == 0),
                                stop=(dc == NDC - 1),
                            )
                for j in range(NJ):
                    for q in range(NTQ):
                        _evac(
                            qt_sb[:, j, ts(q, 512)],
                            pp[j * NTQ + q][:],
                            bq_sb,
                            j,
                            j * NTQ + q,
                        )

        def _stage23():
            # Per head: tkc-outer. E streams as [128, T] fp16 tiles (deep
            # pool => DMA prefetch runs ahead); exp on ACT, mul + norm on DVE.
            with (
                tc.tile_pool(name="sps", bufs=2, space="PSUM") as sps,
                tc.tile_pool(name="ops", bufs=2, space="PSUM") as ops,
                tc.tile_pool(name="epool", bufs=3) as epool,
                tc.tile_pool(name="expool", bufs=4) as expool,
                tc.tile_pool(name="xpool", bufs=6) as xpool,
                tc.tile_pool(name="cpool", bufs=2) as cpool,
                tc.tile_pool(name="npool", bufs=2) as npool,
            ):
                def _avs(h, otiles, tkc, pps):
                    # AV matmuls for tkc, issued one step behind the
                    # S/exp/mul chain so the in-order PE never stalls.
                    for tqb in range(NTQB):
                        for q2 in range(TQB // 512):
                            nc.tensor.matmul(
                                otiles[tqb][:, ts(q2, 512)],
                                lhsT=vo_sb[:, tkc, h * 128 : (h + 1) * 128],
                                rhs=pps[tqb][:, ts(q2, 512)],
                                start=(tkc == 0),
                                stop=(tkc == NTK - 1),
                            )

                def _norm_one(hh, cop, tqb):
                    # divide numerators by the denominator rows of the
                    # evacuated AV tile; runs off the head-transition path
                    jj, hpp = hh // 2, 64 * (hh % 2)
                    tq_sl = slice(tqb * TQB, (tqb + 1) * TQB)
                    rep = npool.tile([64, TQB], F32, tag="rep")
                    if dve_norm:
                        den = npool.tile([64, TQB], F32, tag="den")
                        nc.vector.tensor_copy(den[:], cop[64:128, :])
                        nc.vector.reciprocal_approx_fast(rep[:], den[:])
                    else:
                        ln_t = npool.tile([64, TQB], F32, tag="ln")
                        nc.scalar.activation(ln_t[:], cop[64:128, :], Ln)
                        nc.scalar.activation(rep[:], ln_t[:], Exp, scale=-1.0)
                    nc.vector.tensor_mul(
                        ot_sb[hpp : hpp + 64, jj, tq_sl],
                        cop[0:DH, :],
                        rep[:],
                    )

                deferred = []   # (head, cop, tqb) pairs awaiting divide
                for h in range(HPC):
                    j, hp = h // 2, 64 * (h % 2)
                    otiles = [
                        ops.tile([128, TQB], F32, tag="o", name=f"o{h}_{i}")
                        for i in range(NTQB)
                    ]
                    pending = None
                    eview = d_E.ap()[h].rearrange("(a p) t -> p a t", p=128)
                    et = None
                    for tkc in range(NTK):
                        if tkc % 2 == 0:
                            et = epool.tile([128, 2, T], FP16, tag="e")
                            nc.sync.dma_start(
                                et[:], eview[:, tkc : tkc + 2, :]
                            )
                        cur = []
                        for tqb in range(NTQB):
                            spt = sps.tile([128, TQB], F32, tag="s")
                            for q2 in range(TQB // 512):
                                nc.tensor.matmul(
                                    spt[:, ts(q2, 512)],
                                    lhsT=kt_sb[hp : hp + 64, j, ts(tkc, 128)],
                                    rhs=qt_sb[
                                        hp : hp + 64, j, tqb * TQB + q2 * 512 :
                                        tqb * TQB + (q2 + 1) * 512
                                    ],
                                    start=True,
                                    stop=True,
                                )
                            ex = expool.tile([128, TQB], FP16, tag="ex")
                            nc.scalar.activation(ex[:], spt[:], Exp)
                            pp = xpool.tile([128, TQB], FP16, tag="pp")
                            nc.vector.tensor_mul(
                                pp[:], ex[:], et[:, tkc % 2, ts(tqb, TQB)]
                            )
                            cur.append(pp)
                        if tkc in (3, 9) and deferred:
                            _norm_one(*deferred.pop(0))
                        if av_pipe:
                            if pending is not None:
                                _avs(h, otiles, tkc - 1, pending)
                            pending = cur
                        else:
                            _avs(h, otiles, tkc, cur)
                    if av_pipe:
                        _avs(h, otiles, NTK - 1, pending)
                    # Eagerly evacuate both AV psum tiles (numerator rows
                    # 0:64 + denominator rows 64:128): the next head's AV
                    # matmuls get the banks back after two DVE copies, and
                    # the divides are deferred into the next head's loop
                    # (the in-order DVE queue must not delay its pp muls).
                    for tqb in range(NTQB):
                        cop = cpool.tile([128, TQB], F32, tag="cop")
                        nc.vector.tensor_copy(cop[:], otiles[tqb][:])
                        deferred.append((h, cop, tqb))
                for item in deferred:
                    _norm_one(*item)

        def _stage4():
            # out.T[dm, t] = sum_j wo[:, j, dm].T @ ot[:, j, t]; wo chunk is
            # the stationary operand, reused across all T columns.
            with (
                tc.tile_pool(name="pops", bufs=8, space="PSUM") as pops,
                tc.tile_pool(name="outst", bufs=2) as outst,
            ):
                for dmc in range(DM // 128):
                    ost = outst.tile([128, T], FP16, tag="ost")
                    pos = [
                        pops.tile([128, 512], F32, tag="po", name=f"po{dmc}_{i}")
                        for i in range(T // 512)
                    ]
                    for j in range(NJ):
                        for tt in range(T // 512):
                            nc.tensor.matmul(
                                pos[tt][:],
                                lhsT=wo_sb[:, j, ts(dmc, 128)],
                                rhs=ot_sb[:, j, ts(tt, 512)],
                                start=(j == 0),
                                stop=(j == NJ - 1),
                            )
                    for tt in range(T // 512):
                        # split evacs across ACT and DVE
                        if tt % 2 == 0:
                            nc.scalar.activation(
                                ost[:, ts(tt, 512)],
                                pos[tt][:],
                                mybir.ActivationFunctionType.Copy,
                            )
                        else:
                            nc.vector.tensor_copy(ost[:, ts(tt, 512)], pos[tt][:])
                    nc.sync.dma_start(d_out.ap()[ts(dmc, 128), :], ost[:])

        enabled = set(parts.split(",")) if parts != "all" else {"s1", "s2", "op"}

        def _compute_body():
            if "s1" in enabled:
                _proj_v()
                _proj_q()
                _proj_k()
            if "s2" in enabled:
                _stage23()
            if "op" in enabled:
                _stage4()

        if reps > 1:
            hints = (
                mybir.EngineType.PE,
                mybir.EngineType.Activation,
                mybir.EngineType.DVE,
                mybir.EngineType.SP,
                mybir.EngineType.Pool,
            )
            with tc.For_i(0, reps, 1, hint_engines=hints):
                _compute_body()
        else:
            _compute_body()

    nc.compile()
    return nc


def _pack_w(wT, ndc):
    """[DM, m] (contraction-major) -> [128, ndc*m] SBUF-layout prepack."""
    dm, m = wT.shape
    return np.ascontiguousarray(
        wT.reshape(ndc, 128, m).transpose(1, 0, 2).reshape(128, ndc * m)
    )


def make_in_maps(hidden_q, hidden_kv, mask, position_bias, wq, bq, wk, bk, wv, wo):
    """Host-side sharding/layout prep. Returns per-core input maps."""
    f16 = np.float16
    T = hidden_q.shape[1]
    DM = hidden_q.shape[2]
    NDC = DM // 128
    NJ = M // 128
    hqT = np.ascontiguousarray(hidden_q[0].T, dtype=f16)
    hkvT = hidden_kv[0].T.astype(f16)                       # [DM, T]
    hkvP = _pack_w(hkvT, NDC)                               # [128, NDC*T]
    maskf = mask[0].astype(np.float32)  # [tq, tk]
    in_maps = []
    for c in range(N_CORES):
        sl = slice(c * M, (c + 1) * M)
        wqT = (wq[sl] * (1.0 / np.sqrt(DH))).T.astype(f16)  # [DM, M]
        wkT = wk[sl].T.astype(f16)
        wvT = wv[sl].T.astype(f16)
        woT = wo[:, sl].T.astype(f16)                        # [M, DM]
        pb_c = position_bias[0, c * HPC : (c + 1) * HPC]     # [HPC, tq, tk]
        E = np.exp(pb_c, dtype=np.float32) * (maskf[None] * E_SCALE)
        E = np.ascontiguousarray(E.transpose(0, 2, 1)).astype(f16)
        in_maps.append(
            {
                "hqT": hqT,
                "hkvP": hkvP,
                "wqT": _pack_w(wqT, NDC),
                "wkT": _pack_w(wkT, NDC),
                "wvT": _pack_w(wvT, NDC),
                "woT": _pack_w(woT, NJ),
                "bq": np.ascontiguousarray(bq[sl] * (1.0 / np.sqrt(DH))),
                "bk": np.ascontiguousarray(bk[sl]),
                "E": E,
            }
        )
    return in_maps


def kernel(hidden_q, hidden_kv, mask, position_bias, wq, bq, wk, bk, wv, bv, wo, bo):
    hidden_q = np.asarray(hidden_q, np.float32)
    hidden_kv = np.asarray(hidden_kv, np.float32)
    mask = np.asarray(mask)
    position_bias = np.asarray(position_bias, np.float32)
    wq, bq = np.asarray(wq, np.float32), np.asarray(bq, np.float32)
    wk, bk = np.asarray(wk, np.float32), np.asarray(bk, np.float32)
    wv, bv = np.asarray(wv, np.float32), np.asarray(bv, np.float32)
    wo, bo = np.asarray(wo, np.float32), np.asarray(bo, np.float32)

    T = hidden_q.shape[1]
    DM = hidden_q.shape[2]

    with_bias = bool(np.any(bq) or np.any(bk))
    nc = build_attention_nc(T, DM, with_bias=with_bias)
    in_maps = make_in_maps(
        hidden_q, hidden_kv, mask, position_bias, wq, bq, wk, bk, wv, wo
    )
    res = run_bass_kernel_spmd(nc, in_maps, core_ids=list(range(N_CORES)))
    if res.exec_time_ns is not None:
        print(f"HW exec time: {res.exec_time_ns} ns")

    out = np.zeros((DM, T), np.float64)
    for c in range(N_CORES):
        out += res.results[c]["out"].astype(np.float64)
    out = out.T
    out += (bv.astype(np.float64) @ wo.T.astype(np.float64)) + bo.astype(np.float64)
    return out[None].astype(np.float32)
